# revision 12
# baseline (speedup 1.0000x reference)
"""Trainium2 Bass kernel for nn_Block_25409026523806 (moe_routing).

Transformer block: x = x + attn(rmsnorm(x)); x = x + moe(rmsnorm(x)).
B=4, S=1024, D=1024, H=16 heads (hd=64), ED=4096, fp32 I/O.

Sharding: 8 cores = 4 batches x 2 token-halves. Core c handles batch c//2 and
tokens [h*512, h*512+512) with h = c%2. Each core recomputes K/V for the whole
batch (keys are needed causally), so no cross-core communication is required.
Host reorders tokens to [own | partner] so the query block is always [0:512);
the causal structure is carried by a per-core 0/1 mask input.

All activations stay feature-major ("T-layout", [feat, tok]) so chained
matmuls need no transposes: Y^T = W^T-as-lhsT @ X^T. V is produced directly in
token-major layout by swapping matmul roles, which is what attn@V needs. An
appended ones-column on V yields softmax row-sums in the same matmul.

The MoE gate is skipped: top-k softmax weights renormalized by their own sum
always add to 1 (within 4e-9), so the expert scale is identity.

Matmuls run in bf16 (fp32 accumulation in PSUM); residual path in fp32.
SBUF is managed with a single arena pool whose tags chain tensors with
disjoint lifetimes (Tile inserts the WAR deps on slot reuse).
"""

import numpy as np
import ml_dtypes

import concourse.bass as bass
import concourse.tile as tile
import concourse.mybir as mybir
from concourse import bacc
from concourse.bass_utils import run_bass_kernel_spmd

F32 = mybir.dt.float32
BF16 = mybir.dt.bfloat16
AF = mybir.ActivationFunctionType
BF16NP = ml_dtypes.bfloat16

P = 128
D = 1024
S = 1024          # tokens per batch
TQ = 512          # own tokens per core
B = 4
H = 16
HD = 64
ED = 4096
KT = D // P       # 8 k-tiles over D
MU = ED // P      # 32 ed-tiles
EPS = 1e-6
N_CORES = 8


def build_bass(repeat: int = 1) -> bass.Bass:
    # Bacc's compile() splits multi-waits into EventSemaphore chains, which
    # this walrus build requires (it rejects >1 sync wait per instruction).
    nc = bacc.Bacc()

    xT_d = nc.dram_tensor("xT", [P, KT, S], BF16, kind="ExternalInput")
    xq_d = nc.dram_tensor("xq", [P, KT, TQ], F32, kind="ExternalInput")
    mask_d = nc.dram_tensor("maskT", [P, KT, TQ], BF16, kind="ExternalInput")
    qkw_d = nc.dram_tensor("qkw", [16, P, KT, P], BF16, kind="ExternalInput")
    vw_d = nc.dram_tensor("vw", [P, KT, D], BF16, kind="ExternalInput")
    ow_d = nc.dram_tensor("ow", [P, KT, D], BF16, kind="ExternalInput")
    upw_d = nc.dram_tensor("upw", [MU, P, KT, P], BF16, kind="ExternalInput")
    dww_d = nc.dram_tensor("dww", [KT, P, MU, P], BF16, kind="ExternalInput")
    out_d = nc.dram_tensor("outT", [P, KT, TQ], F32, kind="ExternalOutput")

    with tile.TileContext(nc) as tc:
        with tc.tile_pool(name="arena", bufs=1) as arena, \
             tc.tile_pool(name="psum", bufs=8, space="PSUM") as psp, \
             tc.tile_pool(name="wstream", bufs=3) as wstream, \
             tc.tile_pool(name="probs", bufs=10) as probs_pool, \
             tc.tile_pool(name="scratch", bufs=2) as scratch, \
             tc.tile_pool(name="dwp", bufs=2) as dw_pool, \
             tc.tile_pool(name="outp", bufs=2) as out_pool, \
             tc.tile_pool(name="dbounce", bufs=3, space="DRAM") as db_pool:
            ones_sb = arena.tile([P, 1], BF16, tag="ones", name="ones_sb")
            nc.vector.memset(ones_sb, 1.0)
            eps_sb = arena.tile([1, 1], F32, tag="eps", name="eps_sb")
            nc.vector.memset(eps_sb, EPS)

            for r in range(repeat):
                _emit_block(nc, tc, arena, psp, wstream, probs_pool, scratch,
                            dw_pool, out_pool, db_pool, ones_sb, eps_sb,
                            xT_d, xq_d, mask_d, qkw_d, vw_d, ow_d, upw_d,
                            dww_d, out_d, r)
    nc.compile()
    return nc


def _emit_block(nc, tc, arena, psp, wstream, probs_pool, scratch,
                dw_pool, out_pool, db_pool, ones_sb, eps_sb,
                xT_d, xq_d, mask_d, qkw_d, vw_d, ow_d, upw_d, dww_d, out_d,
                rep):
    def ps_tile(name):
        return psp.tile([P, 512], F32, tag="ps", name=f"{name}_{rep}")

    def at(shape, dtype, tag, name):
        return arena.tile(shape, dtype, tag=tag, name=f"{name}_{rep}")

    # Arena tag chains (disjoint lifetimes share a slot):
    #   t16a: xT -> kT -> h1T          (16 KB/partition)
    #   t16d: xsq -> xnT -> attn -> sq2
    #   t33:  v -> uT                  (32 KB)
    #   t8a:  qT -> hnT
    #   t8b:  mask -> r2b
    #   town: ow;  txq: xq;  tvw: vw   (16 KB each)

    # ---------------- phase 1: loads + rmsnorm1 ----------------
    xT_sb = at([P, KT, S], BF16, "t16a", "xT_sb")
    nc.sync.dma_start(out=xT_sb, in_=xT_d[:, :, :])
    mask_sb = at([P, KT, TQ], BF16, "t8b", "mask_sb")
    nc.sync.dma_start(out=mask_sb, in_=mask_d[:, :, :])
    xq_sb = at([P, KT, TQ], F32, "txq", "xq_sb")
    nc.sync.dma_start(out=xq_sb, in_=xq_d[:, :, :])
    vw_sb = at([P, KT, D], BF16, "tvw", "vw_sb")
    nc.sync.dma_start(out=vw_sb, in_=vw_d[:, :, :])

    xsq_sb = at([P, KT, S], BF16, "t16d", "xsq_sb")
    for kt in range(KT):
        nc.vector.tensor_mul(xsq_sb[:, kt, :], xT_sb[:, kt, :], xT_sb[:, kt, :])

    s1row = at([1, S], F32, "s1row", "s1row")
    r1row = at([1, S], F32, "r1row", "r1row")
    for c in range(S // 512):
        ps = ps_tile(f"ms1_{c}")
        for kt in range(KT):
            nc.tensor.matmul(ps[0:1, :], ones_sb,
                             xsq_sb[:, kt, c * 512:(c + 1) * 512],
                             start=(kt == 0), stop=(kt == KT - 1))
        # sqrt(mean(x^2) + eps), then reciprocal
        nc.scalar.activation(s1row[0:1, c * 512:(c + 1) * 512], ps[0:1, :],
                             AF.Sqrt, bias=eps_sb[0:1, 0:1], scale=1.0 / D)
        nc.vector.reciprocal(r1row[0:1, c * 512:(c + 1) * 512],
                             s1row[0:1, c * 512:(c + 1) * 512])
    r1b = scratch.tile([P, S], F32, tag="rec", name=f"r1b_{rep}")
    db1 = db_pool.tile([1, S], F32, tag="db", name=f"db1_{rep}")
    nc.sync.dma_start(out=db1, in_=r1row[0:1, :])
    nc.sync.dma_start(out=r1b, in_=db1.to_broadcast([P, S]))

    xnT_sb = at([P, KT, S], BF16, "t16d", "xnT_sb")
    for kt in range(KT):
        nc.vector.tensor_mul(xnT_sb[:, kt, :], xT_sb[:, kt, :], r1b)

    # ---------------- phase 2: qkv projections ----------------
    qT_sb = at([P, KT, TQ], BF16, "t8a", "qT_sb")       # q, own tokens
    kT_sb = at([P, KT, S], BF16, "t16a", "kT_sb")       # k, all tokens
    v_sb = at([P, KT, H, HD + 1], BF16, "t33", "v_sb")  # v + ones col
    nc.vector.memset(v_sb[:, :, :, HD:HD + 1], 1.0)

    for m in range(16):  # q (0-7, own tokens) and k (8-15, all tokens)
        wt = wstream.tile([P, KT, P], BF16, tag="qkw", name=f"qkw_{m}_{rep}")
        nc.sync.dma_start(out=wt, in_=qkw_d[m, :, :, :])
        n_chunks = 1 if m < KT else 2
        for n in range(n_chunks):
            ps = ps_tile(f"qk_{m}_{n}")
            for kt in range(KT):
                nc.tensor.matmul(ps, wt[:, kt, :],
                                 xnT_sb[:, kt, n * 512:(n + 1) * 512],
                                 start=(kt == 0), stop=(kt == KT - 1))
            if m < KT:
                nc.scalar.copy(qT_sb[:, m, :], ps)
            else:
                nc.scalar.copy(kT_sb[:, m - KT, n * 512:(n + 1) * 512], ps)
    # v in token-major layout: lhsT = xnT (tokens as M), rhs = v-weights
    for tokt in range(KT):
        for n in range(2):
            ps = ps_tile(f"v_{tokt}_{n}")
            for kt in range(KT):
                nc.tensor.matmul(ps, xnT_sb[:, kt, tokt * P:(tokt + 1) * P],
                                 vw_sb[:, kt, n * 512:(n + 1) * 512],
                                 start=(kt == 0), stop=(kt == KT - 1))
            nc.vector.tensor_copy(
                out=v_sb[:, tokt, n * 8:(n + 1) * 8, 0:HD],
                in_=ps.rearrange("p (a b) -> p a b", a=8))

    ow_sb = at([P, KT, D], BF16, "town", "ow_sb")
    nc.sync.dma_start(out=ow_sb, in_=ow_d[:, :, :])

    # ---------------- phase 3: attention ----------------
    attn_sb = at([P, KT, TQ], BF16, "t16d", "attn_sb")
    for t in range(KT):  # head pairs (2t, 2t+1)
        pbs = []
        for hh in range(2):
            lo, hi = hh * HD, (hh + 1) * HD
            pb_k = []
            for kt in range(KT):
                pb = probs_pool.tile([P, TQ], BF16, tag="probs",
                                     name=f"probs_{t}_{hh}_{kt}_{rep}")
                ps = ps_tile(f"sc_{t}_{hh}_{kt}")
                nc.tensor.matmul(ps, kT_sb[lo:hi, t, kt * P:(kt + 1) * P],
                                 qT_sb[lo:hi, t, :], start=True, stop=True)
                nc.scalar.activation(pb, ps, AF.Exp, scale=0.125)
                nc.vector.tensor_mul(pb, pb, mask_sb[:, kt, :])
                pb_k.append(pb)
            pbs.append(pb_k)
        psA = ps_tile(f"avA_{t}")
        psB = ps_tile(f"avB_{t}")
        for kt in range(KT):
            nc.tensor.matmul(psA[0:HD + 1, :], v_sb[:, kt, 2 * t, :],
                             pbs[0][kt], start=(kt == 0), stop=(kt == KT - 1))
            nc.tensor.matmul(psB[0:HD + 1, :], v_sb[:, kt, 2 * t + 1, :],
                             pbs[1][kt], start=(kt == 0), stop=(kt == KT - 1))
        rec = scratch.tile([P, 2 * TQ], F32, tag="rec", name=f"rec_{t}_{rep}")
        nc.vector.reciprocal(rec[HD:HD + 1, 0:TQ], psA[HD:HD + 1, :])
        nc.vector.reciprocal(rec[HD:HD + 1, TQ:2 * TQ], psB[HD:HD + 1, :])
        rb = scratch.tile([HD, 2 * TQ], F32, tag="rb", name=f"rb_{t}_{rep}")
        dbr = db_pool.tile([1, 2 * TQ], F32, tag="db", name=f"dbr_{t}_{rep}")
        nc.sync.dma_start(out=dbr, in_=rec[HD:HD + 1, :])
        nc.sync.dma_start(out=rb[:, 0:TQ],
                          in_=dbr[0:1, 0:TQ].to_broadcast([HD, TQ]))
        nc.sync.dma_start(out=rb[:, TQ:2 * TQ],
                          in_=dbr[0:1, TQ:2 * TQ].to_broadcast([HD, TQ]))
        nc.vector.tensor_mul(attn_sb[0:HD, t, :], psA[0:HD, :], rb[:, 0:TQ])
        scrB = scratch.tile([HD, TQ], BF16, tag="scrB", name=f"scrB_{t}_{rep}")
        nc.vector.tensor_mul(scrB, psB[0:HD, :], rb[:, TQ:2 * TQ])
        nc.sync.dma_start(out=attn_sb[HD:P, t, :], in_=scrB)

    # ---------------- phase 4: o-proj + residual + rmsnorm2 ----------------
    h1T_sb = at([P, KT, TQ], F32, "t16a", "h1T_sb")
    for m in range(KT):
        ps = ps_tile(f"o_{m}")
        for kt in range(KT):
            nc.tensor.matmul(ps, ow_sb[:, kt, m * P:(m + 1) * P],
                             attn_sb[:, kt, :], start=(kt == 0), stop=(kt == KT - 1))
        nc.vector.tensor_add(out=h1T_sb[:, m, :], in0=ps, in1=xq_sb[:, m, :])

    sq2_sb = at([P, KT, TQ], BF16, "t16d", "sq2_sb")
    for m in range(KT):
        nc.vector.tensor_mul(sq2_sb[:, m, :], h1T_sb[:, m, :], h1T_sb[:, m, :])
    s2row = at([1, TQ], F32, "s1row", "s2row")
    r2row = at([1, TQ], F32, "r1row", "r2row")
    ps = ps_tile("ms2")
    for m in range(KT):
        nc.tensor.matmul(ps[0:1, :], ones_sb, sq2_sb[:, m, :],
                         start=(m == 0), stop=(m == KT - 1))
    nc.scalar.activation(s2row[0:1, :], ps[0:1, :], AF.Sqrt,
                         bias=eps_sb[0:1, 0:1], scale=1.0 / D)
    nc.vector.reciprocal(r2row[0:1, :], s2row[0:1, :])
    r2b = at([P, TQ], F32, "t8b", "r2b")
    db2 = db_pool.tile([1, TQ], F32, tag="db", name=f"db2_{rep}")
    nc.sync.dma_start(out=db2, in_=r2row[0:1, :])
    nc.sync.dma_start(out=r2b, in_=db2.to_broadcast([P, TQ]))
    hnT_sb = at([P, KT, TQ], BF16, "t8a", "hnT_sb")
    for m in range(KT):
        nc.vector.tensor_mul(hnT_sb[:, m, :], h1T_sb[:, m, :], r2b)

    # ---------------- phase 5: MoE (shared expert; gate == identity) -------
    uT_sb = at([P, MU, TQ], BF16, "t33", "uT_sb")
    for m in range(MU):
        wt = wstream.tile([P, KT, P], BF16, tag="upw", name=f"upw_{m}_{rep}")
        nc.sync.dma_start(out=wt, in_=upw_d[m, :, :, :])
        ps = ps_tile(f"up_{m}")
        for kt in range(KT):
            nc.tensor.matmul(ps, wt[:, kt, :], hnT_sb[:, kt, :],
                             start=(kt == 0), stop=(kt == KT - 1))
        nc.scalar.activation(uT_sb[:, m, :], ps, AF.Silu)

    for m in range(KT):
        dw = dw_pool.tile([P, MU, P], BF16, tag="dw", name=f"dw_{m}_{rep}")
        nc.sync.dma_start(out=dw, in_=dww_d[m, :, :, :])
        ps = ps_tile(f"dn_{m}")
        for kt in range(MU):
            nc.tensor.matmul(ps, dw[:, kt, :], uT_sb[:, kt, :],
                             start=(kt == 0), stop=(kt == MU - 1))
        ot = out_pool.tile([P, TQ], F32, tag="ot", name=f"ot_{m}_{rep}")
        nc.vector.tensor_add(out=ot, in0=ps, in1=h1T_sb[:, m, :])
        nc.sync.dma_start(out=out_d[:, m, :], in_=ot)


# ---------------------------------------------------------------------------
# Host side
# ---------------------------------------------------------------------------

_NC_CACHE: dict = {}


def _get_nc(repeat: int = 1):
    if repeat not in _NC_CACHE:
        _NC_CACHE[repeat] = build_bass(repeat)
    return _NC_CACHE[repeat]


def _tile_k(a: np.ndarray) -> np.ndarray:
    """[K, M] -> [128, K//128, M] partition-major tiling."""
    K, M = a.shape
    return np.ascontiguousarray(a.reshape(K // P, P, M).transpose(1, 0, 2))


def _stream_tiles(a: np.ndarray) -> np.ndarray:
    """[K, M] -> [M//128, 128, K//128, 128]: per-m-tile contiguous blocks."""
    t = _tile_k(a)                       # [128, kt, M]
    K, M = a.shape
    return np.ascontiguousarray(
        t.reshape(P, K // P, M // P, P).transpose(2, 0, 1, 3))


def _prep_shared(n1_w, qkv_w, o_w, n2_w, up_w, down_w):
    qkvw_full = (qkv_w * n1_w[None, :]).T.astype(BF16NP)   # [D, 3D]
    qkw = _stream_tiles(qkvw_full[:, :2 * D])              # [16,128,8,128]
    vw = _tile_k(qkvw_full[:, 2 * D:])                     # [128,8,1024]
    ow = _tile_k(o_w.T.astype(BF16NP))
    upw = _stream_tiles((up_w * n2_w[None, :]).T.astype(BF16NP))  # [32,...]
    # down: [8, 128, 32, 128]: dww[m, p, kt, n] = down_w[m*128+n, kt*128+p]
    dww = np.ascontiguousarray(
        down_w.astype(BF16NP).reshape(KT, P, MU, P).transpose(0, 3, 2, 1))
    return qkw, vw, ow, upw, dww


def _make_in_maps(x, n1_w, qkv_w, o_w, n2_w, gate_w, up_w, down_w):
    qkw, vw, ow, upw, dww = _prep_shared(n1_w, qkv_w, o_w, n2_w, up_w, down_w)
    in_maps = []
    for c in range(N_CORES):
        b, h = divmod(c, 2)
        own = np.arange(h * TQ, (h + 1) * TQ)
        other = np.arange((1 - h) * TQ, (2 - h) * TQ)
        perm = np.concatenate([own, other])
        xT = np.ascontiguousarray(x[b][perm].T)          # [D, S] f32
        xT_t = _tile_k(xT)                               # [128, 8, 1024]
        xq_t = np.ascontiguousarray(xT_t[:, :, :TQ])
        allowed = (perm[:, None] <= own[None, :])        # [S keys, TQ queries]
        maskT = np.ascontiguousarray(
            allowed.reshape(KT, P, TQ).transpose(1, 0, 2)).astype(BF16NP)
        in_maps.append({
            "xT": xT_t.astype(BF16NP), "xq": xq_t, "maskT": maskT,
            "qkw": qkw, "vw": vw, "ow": ow, "upw": upw, "dww": dww,
        })
    return in_maps


def _run(in_maps, repeat: int = 1):
    nc = _get_nc(repeat)
    return run_bass_kernel_spmd(nc, in_maps, core_ids=list(range(N_CORES)))


def kernel(x, n1_w, qkv_w, o_w, n2_w, gate_w, up_w, down_w):
    x = np.asarray(x, dtype=np.float32)
    args = [np.asarray(a, dtype=np.float32)
            for a in (n1_w, qkv_w, o_w, n2_w, gate_w, up_w, down_w)]
    in_maps = _make_in_maps(x, *args)
    res = _run(in_maps)
    out = np.empty((B, S, D), np.float32)
    for c in range(N_CORES):
        b, h = divmod(c, 2)
        outT = res.results[c]["outT"]                    # [128, 8, 512]
        out[b, h * TQ:(h + 1) * TQ] = (
            outT.transpose(1, 0, 2).reshape(D, TQ).T)
    return out
